# revision 30
# baseline (speedup 1.0000x reference)
"""Llama-style transformer block on 8 TRN2 NeuronCores.

v8: skew-immune design.  Cross-core launch skew (~25-70us) makes any
engine-FIFO instruction that waits on a collective a head-of-line hazard
(the tile scheduler hoists aggressively and does not model peer skew).
So v8 keeps ONLY the unavoidable big collectives (RS of wo partials, AG
of h, RS of FFN partials) and computes everything else locally:
  - x RMS stats: full sum-of-squares from the replicated x_ch tiles via
    ones-matmuls (x is replicated on every core anyway).  No AllReduce.
  - FFN RMS stats: from the gathered hn tiles via ones-matmuls, computed
    at each FFN chunk start (prefetched a chunk ahead).  No AllReduce.
  - h_block / residual are single fat DMAs + one wide DVE op, minimizing
    the number of FIFO slots that can block on an RS result.
  - wo ROW-sharded (no attnT AllGather); attention rowsums staggered one
    ktile behind scores; AV after rowsum chain so recip hides under it.
Program: A0..A3 | B0 B1 wo0 B2 wo1 B3 wo2 h0 wo3 h1 |
         ffn0[hn,stats,scale,ft,h2,h3,w2,RSf0] ffn1[...,res0,...] ...
"""

import math

import ml_dtypes
import numpy as np

import concourse.bass as bass
import concourse.mybir as mybir
import concourse.tile as tile
from concourse import bacc
from concourse.bass_utils import run_bass_kernel_spmd

S = 2048
D = 4096
HD = 128
NH = 32
F = 11008
CORES = 8
NHC = NH // CORES          # heads per core = 4
DQ = NHC * HD              # q/k/v dims per core = 512
FC = F // CORES            # ffn dims per core = 1376
FT = 11                    # padded f-tiles per core
FP = FT * 128
EPS = 1e-5
P = 128
NCH = 4                    # 512-token chunks
CW = S // NCH              # chunk width = 512
DT = D // P                # d tiles = 32
ST = S // P                # s tiles = 16

CDT = mybir.dt.bfloat16
NP_CDT = ml_dtypes.bfloat16

_COMPILED = None


def _build():
    nc = bacc.Bacc("TRN2", target_bir_lowering=False, debug=False,
                   num_devices=CORES)
    f32 = mybir.dt.float32

    # ---- kernel I/O ----
    xT_s = nc.declare_dram_parameter("xT_s", [DQ, S], f32, isOutput=False)
    x_ch = nc.declare_dram_parameter("x_ch", [NCH, P, DT, CW], CDT,
                                     isOutput=False)
    w_qk = nc.declare_dram_parameter("w_qk", [8, P, DT, P], CDT, isOutput=False)
    w_v = nc.declare_dram_parameter("w_v", [P, DT, DQ], CDT, isOutput=False)
    w_o = nc.declare_dram_parameter("w_o", [P, DT, NHC, P], CDT, isOutput=False)
    w_1 = nc.declare_dram_parameter("w_1", [FT, P, DT, P], CDT, isOutput=False)
    w_3 = nc.declare_dram_parameter("w_3", [FT, P, DT, P], CDT, isOutput=False)
    w_2 = nc.declare_dram_parameter("w_2", [32, P, FT, P], CDT, isOutput=False)
    cos2 = nc.declare_dram_parameter("cos2", [P, S], CDT, isOutput=False)
    sinsg2 = nc.declare_dram_parameter("sinsg2", [P, S], CDT, isOutput=False)
    dmask = nc.declare_dram_parameter("dmask", [P, P], f32, isOutput=False)
    outT_s = nc.declare_dram_parameter("outT_s", [DQ, S], f32, isOutput=True)

    # ---- internal DRAM ----
    s1row = nc.dram_tensor("s1row", [1, S], f32)
    oT_cc = [nc.dram_tensor(f"oT_cc{c}", [D, CW], CDT) for c in range(NCH)]
    o_rs = [nc.dram_tensor(f"o_rs{c}", [DQ, CW], CDT) for c in range(NCH)]
    h_cc = [nc.dram_tensor(f"h_cc{c}", [DQ, CW], CDT) for c in range(NCH)]
    hT_ag = [nc.dram_tensor(f"hT_ag{c}", [D, CW], CDT, addr_space="Shared")
             for c in range(NCH)]
    foT_cc = [nc.dram_tensor(f"foT_cc{c}", [D, CW], CDT) for c in range(NCH)]
    fo_rs = [nc.dram_tensor(f"fo_rs{c}", [DQ, CW], CDT) for c in range(NCH)]

    RG = [list(range(CORES))]
    ADD = mybir.AluOpType.add
    BYP = mybir.AluOpType.bypass
    EXP = mybir.ActivationFunctionType.Exp
    SQRT = mybir.ActivationFunctionType.Sqrt
    SILU = mybir.ActivationFunctionType.Silu
    ISQ = 1.0 / math.sqrt(HD)

    def ch(c):
        return slice(CW * c, CW * (c + 1))

    with tile.TileContext(nc) as tc:
        with (
            tc.tile_pool(name="persist", bufs=1) as persist,
        ):
            ones = persist.tile([P, 1], CDT)
            nc.vector.memset(ones[:], 1.0)
            eps_sb = persist.tile([P, 1], f32)
            nc.vector.memset(eps_sb[:], EPS)
            dmask_sb = persist.tile([P, P], f32)
            nc.gpsimd.dma_start(out=dmask_sb[:], in_=dmask[:])
            hTb = persist.tile([P, NHC, S], CDT)
            s1tok = persist.tile([P, ST], f32)

            with tc.tile_pool(name="qkvsb", bufs=1) as qkvsb:
                qts = [qkvsb.tile([P, S], CDT, tag=f"qt{h}", name=f"qt{h}")
                       for h in range(NHC)]
                kts = [qkvsb.tile([P, S], CDT, tag=f"kt{h}", name=f"kt{h}")
                       for h in range(NHC)]
                v_sb = qkvsb.tile([P, ST, DQ], CDT)

                # ======== stage A: local stats + Q/K/V (+RoPE) ========
                with (
                    tc.tile_pool(name="tbl", bufs=1) as tbl,
                    tc.tile_pool(name="xst1", bufs=2) as xst1,
                    tc.tile_pool(name="xst", bufs=2) as xst,
                    tc.tile_pool(name="stAx", bufs=5) as stAx,
                    tc.tile_pool(name="stAw", bufs=2) as stAw,
                    tc.tile_pool(name="wvp", bufs=1) as wvp,
                    tc.tile_pool(name="rope", bufs=2) as rope,
                    tc.tile_pool(name="ps_qkv", bufs=3, space="PSUM") as ps_qkv,
                    tc.tile_pool(name="ps_v", bufs=1, space="PSUM") as ps_v,
                    tc.tile_pool(name="ps_xst", bufs=1, space="PSUM") as ps_xst,
                ):
                    cos_raw = tbl.tile([P, S], CDT, tag="cosr")
                    sin_raw = tbl.tile([P, S], CDT, tag="sinr")
                    nc.gpsimd.dma_start(out=cos_raw[:], in_=cos2[:])
                    nc.gpsimd.dma_start(out=sin_raw[:], in_=sinsg2[:])
                    wv_sb = wvp.tile([P, DT, DQ], CDT)
                    nc.scalar.dma_start(out=wv_sb[:], in_=w_v[:])

                    for c in range(NCH):
                      with nc.named_scope(f"qkv_c{c}"):
                        xq = [stAx.tile([P, 8, CW], CDT, tag="xq",
                                        name=f"xq{j}_{c}")
                              for j in range(4)]
                        for j in range(4):
                            nc.gpsimd.dma_start(
                                out=xq[j][:],
                                in_=x_ch[c][:, 8 * j:8 * (j + 1), :])

                        def xkt(kt):
                            return xq[kt // 8][:, kt % 8, :]

                        # --- local RMS stats: full ssq from replicated x ---
                        pst = ps_xst.tile([1, CW], f32, tag="pst")
                        for j in range(4):
                            sqx = xst1.tile([P, 8, CW], CDT, tag="sqx",
                                            name=f"sqx{j}_{c}")
                            nc.vector.tensor_mul(sqx[:], xq[j][:], xq[j][:])
                            for kk in range(8):
                                nc.tensor.matmul(
                                    pst[:], ones[:], sqx[:, kk, :],
                                    start=(j == 0 and kk == 0),
                                    stop=(j == 3 and kk == 7))
                        row = xst.tile([1, CW], f32, tag="xrow",
                                       name=f"xrow{c}")
                        nc.scalar.activation(out=row[:], in_=pst[:],
                                             func=SQRT, bias=eps_sb[0:1],
                                             scale=1.0 / D)
                        nc.vector.reciprocal(out=row[:], in_=row[:])
                        # token-major copy for the V scale
                        nc.gpsimd.dma_start(out=s1row[0:1, ch(c)], in_=row[:])
                        nc.gpsimd.dma_start(
                            out=s1tok[:, 4 * c:4 * c + 4],
                            in_=s1row[0:1, ch(c)].rearrange(
                                "o (j p) -> p (o j)", p=P))
                        s1b = xst.tile([1, CW], CDT, tag="s1b", name=f"s1b{c}")
                        nc.vector.tensor_copy(out=s1b[:], in_=row[:])
                        s1rep = xst.tile([P, CW], CDT, tag="s1rep",
                                         name=f"s1rep{c}")
                        nc.gpsimd.partition_broadcast(s1rep[:], s1b[:])
                        cs_t = rope.tile([P, CW], CDT, tag="cs", name=f"cs{c}")
                        sn_t = rope.tile([P, CW], CDT, tag="sn", name=f"sn{c}")
                        nc.vector.tensor_mul(cs_t[:], cos_raw[:, ch(c)],
                                             s1rep[:])
                        nc.vector.tensor_mul(sn_t[:], sin_raw[:, ch(c)],
                                             s1rep[:])

                        # --- Q and K projections + RoPE ---
                        for ot in range(8):
                            wt = stAw.tile([P, DT, P], CDT, tag="wqk")
                            if ot % 2 == 0:
                                nc.scalar.dma_start(out=wt[:], in_=w_qk[ot])
                            else:
                                nc.sync.dma_start(out=wt[:], in_=w_qk[ot])
                            pt = ps_qkv.tile([P, CW], f32, tag="pqk")
                            for kt in range(DT):
                                nc.tensor.matmul(pt[:], wt[:, kt], xkt(kt),
                                                 start=(kt == 0),
                                                 stop=(kt == DT - 1))
                            swp = rope.tile([P, CW], f32, tag="swp")
                            nc.vector.tensor_copy(swp[0:64, :], pt[64:128, :])
                            nc.vector.tensor_copy(swp[64:128, :], pt[0:64, :])
                            t1 = rope.tile([P, CW], f32, tag="t1")
                            nc.vector.tensor_mul(t1[:], pt[:], cs_t[:])
                            nc.vector.tensor_mul(swp[:], swp[:], sn_t[:])
                            dst = qts[ot % 4] if ot < 4 else kts[ot % 4]
                            nc.vector.tensor_add(dst[:, ch(c)], t1[:], swp[:])

                        # --- V: 4 token-tiles of this chunk ---
                        pts = [ps_v.tile([P, DQ], f32, tag=f"pv{i}",
                                         name=f"pv{i}") for i in range(4)]
                        for kt in range(DT):
                            for i in range(4):
                                tok = slice(P * i, P * (i + 1))
                                nc.tensor.matmul(
                                    pts[i][:], xkt(kt)[:, tok],
                                    wv_sb[:, kt, :],
                                    start=(kt == 0), stop=(kt == DT - 1))
                        for i in range(4):
                            st = 4 * c + i
                            nc.vector.tensor_scalar_mul(
                                out=v_sb[:, st, :], in0=pts[i][:],
                                scalar1=s1tok[:, st:st + 1])

                def h_block(c, hst, xtp):
                    # RS(o_c) must be complete (with skew margin) at the
                    # wall-clock where the gpsimd FIFO reaches this block.
                    with nc.named_scope(f"h_c{c}"):
                        osb4 = hst.tile([P, NHC, CW], CDT, tag="osb")
                        nc.gpsimd.dma_start(
                            out=osb4[:],
                            in_=o_rs[c][:].rearrange("(i p) s -> p i s", p=P))
                        xt4 = xtp.tile([P, NHC, CW], f32, tag="xt")
                        nc.sync.dma_start(
                            out=xt4[:],
                            in_=xT_s[:, ch(c)].rearrange("(i p) s -> p i s",
                                                         p=P))
                        nc.vector.tensor_add(hTb[:, :, ch(c)], xt4[:],
                                             osb4[:])
                        nc.gpsimd.dma_start(
                            out=h_cc[c][:].rearrange("(p k) s -> p k s", p=P),
                            in_=hTb[:, :, ch(c)])
                        nc.gpsimd.collective_compute(
                            "AllGather", BYP, ins=[h_cc[c][:]],
                            outs=[hT_ag[c][:]], replica_groups=RG)

                # ======== stage B: attention + row-sharded wo ========
                with (
                    tc.tile_pool(name="stB", bufs=4) as stB,
                    tc.tile_pool(name="exps", bufs=18) as exps,
                    tc.tile_pool(name="attp", bufs=8) as attp,
                    tc.tile_pool(name="woW", bufs=1) as woW,
                    tc.tile_pool(name="hstB", bufs=2) as hstB,
                    tc.tile_pool(name="xtpB", bufs=2) as xtpB,
                    tc.tile_pool(name="ps_sc", bufs=3, space="PSUM") as ps_sc,
                    tc.tile_pool(name="ps_av", bufs=2, space="PSUM") as ps_av,
                    tc.tile_pool(name="ps_sm", bufs=1, space="PSUM") as ps_sm,
                    tc.tile_pool(name="ps_wo", bufs=2, space="PSUM") as ps_wo,
                ):
                    wo_sb = woW.tile([P, DT, NHC, P], CDT)
                    nc.sync.dma_start(out=wo_sb[:], in_=w_o[:])

                    at_ts = {}

                    def attn_chunk(qc):
                      with nc.named_scope(f"attn_c{qc}"):
                        nkt = 4 * qc + 4
                        for hh in range(NHC):
                            qt, kt_t = qts[hh], kts[hh]
                            smp = ps_sm.tile([1, CW], f32, tag="smp")
                            ets = []
                            for ktile in range(nkt):
                                diag = ktile >= 4 * qc
                                col0 = P * (ktile - 4 * qc) if diag else 0
                                scp = ps_sc.tile([P, CW], f32, tag="scp")
                                nc.tensor.matmul(
                                    scp[:, col0:],
                                    kt_t[:, P * ktile:P * (ktile + 1)],
                                    qt[:, CW * qc + col0:CW * (qc + 1)],
                                    start=True, stop=True)
                                if diag:
                                    nc.vector.tensor_add(
                                        scp[:, col0:col0 + P],
                                        scp[:, col0:col0 + P], dmask_sb[:])
                                et = exps.tile([P, CW], CDT, tag="et")
                                nc.scalar.activation(out=et[:, col0:],
                                                     in_=scp[:, col0:],
                                                     func=EXP, scale=ISQ)
                                ets.append((et, col0))
                                if ktile > 0:
                                    pe, pc0 = ets[ktile - 1]
                                    nc.tensor.matmul(smp[:, pc0:], ones[:],
                                                     pe[:, pc0:],
                                                     start=(ktile == 1),
                                                     stop=False)
                            pe, pc0 = ets[nkt - 1]
                            nc.tensor.matmul(smp[:, pc0:], ones[:],
                                             pe[:, pc0:], start=(nkt == 1),
                                             stop=True)
                            rec = stB.tile([1, CW], f32, tag="rec")
                            nc.vector.reciprocal(out=rec[:], in_=smp[:])
                            avp = ps_av.tile([P, CW], f32, tag="avp")
                            for ktile in range(nkt):
                                et, col0 = ets[ktile]
                                nc.tensor.matmul(
                                    avp[:, col0:],
                                    v_sb[:, ktile, P * hh:P * (hh + 1)],
                                    et[:, col0:], start=(ktile == 0),
                                    stop=(ktile == nkt - 1))
                            rrep = stB.tile([P, CW], f32, tag="rrep")
                            nc.gpsimd.partition_broadcast(rrep[:], rec[:])
                            att = attp.tile([P, CW], CDT, tag="att")
                            nc.vector.tensor_mul(att[:], avp[:], rrep[:])
                            at_ts[(qc, hh)] = att

                    def wo_mm(c):
                      with nc.named_scope(f"wo_c{c}"):
                        for ot in range(DT):
                            pt = ps_wo.tile([P, CW], f32, tag="pwo")
                            for kt in range(NHC):
                                nc.tensor.matmul(pt[:], wo_sb[:, ot, kt, :],
                                                 at_ts[(c, kt)][:],
                                                 start=(kt == 0),
                                                 stop=(kt == NHC - 1))
                            og = stB.tile([P, CW], CDT, tag="og")
                            if c == 3:
                                # exp traffic is over; free the DVE for the
                                # FFN-c0 warmup chain
                                nc.scalar.copy(out=og[:], in_=pt[:])
                            else:
                                nc.vector.tensor_copy(out=og[:], in_=pt[:])
                            nc.sync.dma_start(
                                out=oT_cc[c][P * ot:P * (ot + 1), :],
                                in_=og[:])
                        for kt in range(NHC):
                            del at_ts[(c, kt)]
                        nc.gpsimd.collective_compute(
                            "ReduceScatter", ADD, ins=[oT_cc[c][:]],
                            outs=[o_rs[c][:]], replica_groups=RG)

                    attn_chunk(0)
                    attn_chunk(1)
                    wo_mm(0)
                    attn_chunk(2)
                    wo_mm(1)
                    attn_chunk(3)
                    wo_mm(2)
                    h_block(0, hstB, xtpB)
                    wo_mm(3)
                    h_block(1, hstB, xtpB)

            # ====== stage D: FFN (local stats) + chunked RS + residual ====
            with (
                tc.tile_pool(name="stDh", bufs=2) as stDh,
                tc.tile_pool(name="stDs", bufs=2) as stDs,
                tc.tile_pool(name="stDw", bufs=2) as stDw,
                tc.tile_pool(name="stDw2", bufs=3) as stDw2,
                tc.tile_pool(name="stDg", bufs=2) as stDg,
                tc.tile_pool(name="stDt", bufs=3) as stDt,
                tc.tile_pool(name="hstD", bufs=2) as hstD,
                tc.tile_pool(name="xtpD", bufs=2) as xtpD,
                tc.tile_pool(name="ps_f1", bufs=2, space="PSUM") as ps_f1,
                tc.tile_pool(name="ps_f3", bufs=2, space="PSUM") as ps_f3,
                tc.tile_pool(name="ps_w2", bufs=2, space="PSUM") as ps_w2,
                tc.tile_pool(name="ps_hst", bufs=1, space="PSUM") as ps_hst,
            ):

                def residual(c):
                    # RS(f_c) must be complete at the wall-clock where the
                    # FIFOs reach these three ops.
                    with nc.named_scope(f"res_c{c}"):
                        osb4 = hstD.tile([P, NHC, CW], CDT, tag="osb")
                        nc.gpsimd.dma_start(
                            out=osb4[:],
                            in_=fo_rs[c][:].rearrange("(i p) s -> p i s",
                                                      p=P))
                        res4 = hstD.tile([P, NHC, CW], f32, tag="res")
                        nc.vector.tensor_add(res4[:], hTb[:, :, ch(c)],
                                             osb4[:])
                        nc.sync.dma_start(
                            out=outT_s[:, ch(c)].rearrange(
                                "(i p) s -> p i s", p=P),
                            in_=res4[:])

                for c in range(NCH):
                    with nc.named_scope(f"ffn_c{c}"):
                        hn_sb = stDh.tile([P, DT, CW], CDT, tag="hn")
                        # (k s) merged: each (p, r) is a 4KB contiguous run
                        nc.gpsimd.dma_start(
                            out=hn_sb[:].rearrange("p (r k) s -> p r (k s)",
                                                   r=CORES),
                            in_=hT_ag[c][:].rearrange(
                                "(r p k) s -> p r (k s)", p=P, k=NHC))
                        # local FFN RMS stats from the gathered h
                        psh = ps_hst.tile([1, CW], f32, tag="psh")
                        for kt in range(DT):
                            sq = stDs.tile([P, CW], CDT, tag="sq")
                            nc.vector.tensor_mul(sq[:], hn_sb[:, kt, :],
                                                 hn_sb[:, kt, :])
                            nc.tensor.matmul(psh[:], ones[:], sq[:],
                                             start=(kt == 0),
                                             stop=(kt == DT - 1))
                        rh = stDs.tile([1, CW], f32, tag="hrow")
                        nc.scalar.activation(out=rh[:], in_=psh[:], func=SQRT,
                                             bias=eps_sb[0:1], scale=1.0 / D)
                        nc.vector.reciprocal(out=rh[:], in_=rh[:])
                        s2b = stDs.tile([1, CW], CDT, tag="s2b")
                        nc.vector.tensor_copy(out=s2b[:], in_=rh[:])
                        s2l = stDs.tile([P, CW], CDT, tag="s2l")
                        nc.gpsimd.partition_broadcast(s2l[:], s2b[:])
                        for kt in range(DT):
                            nc.vector.tensor_mul(hn_sb[:, kt, :],
                                                 hn_sb[:, kt, :], s2l[:])
                        g_sb = stDg.tile([P, FT, CW], CDT, tag="g")
                        for ft in range(FT):
                            w1t = stDw.tile([P, DT, P], CDT, tag="w1")
                            w3t = stDw.tile([P, DT, P], CDT, tag="w3")
                            nc.scalar.dma_start(out=w1t[:], in_=w_1[ft])
                            nc.sync.dma_start(out=w3t[:], in_=w_3[ft])
                            p1 = ps_f1.tile([P, CW], f32, tag="p1")
                            p3 = ps_f3.tile([P, CW], f32, tag="p3")
                            for kt in range(DT):
                                nc.tensor.matmul(p1[:], w1t[:, kt],
                                                 hn_sb[:, kt, :],
                                                 start=(kt == 0),
                                                 stop=(kt == DT - 1))
                            for kt in range(DT):
                                nc.tensor.matmul(p3[:], w3t[:, kt],
                                                 hn_sb[:, kt, :],
                                                 start=(kt == 0),
                                                 stop=(kt == DT - 1))
                            tsi = stDt.tile([P, CW], CDT, tag="tsi")
                            nc.scalar.activation(out=tsi[:], in_=p1[:],
                                                 func=SILU)
                            nc.vector.tensor_mul(g_sb[:, ft, :], tsi[:],
                                                 p3[:])
                        if c == 0:
                            h_block(2, hstD, xtpD)
                            h_block(3, hstD, xtpD)
                        else:
                            residual(c - 1)
                        for ot in range(32):
                            w2t = stDw2.tile([P, FT, P], CDT, tag="w2")
                            if ot % 2 == 0:
                                nc.scalar.dma_start(out=w2t[:], in_=w_2[ot])
                            else:
                                nc.sync.dma_start(out=w2t[:], in_=w_2[ot])
                            pt = ps_w2.tile([P, CW], f32, tag="pw2")
                            for ft in range(FT):
                                nc.tensor.matmul(pt[:], w2t[:, ft],
                                                 g_sb[:, ft, :],
                                                 start=(ft == 0),
                                                 stop=(ft == FT - 1))
                            og = stDt.tile([P, CW], CDT, tag="og")
                            if ot % 2 == 0:
                                nc.vector.tensor_copy(out=og[:], in_=pt[:])
                            else:
                                nc.scalar.copy(out=og[:], in_=pt[:])
                            nc.sync.dma_start(
                                out=foT_cc[c][P * ot:P * (ot + 1), :],
                                in_=og[:])
                        nc.gpsimd.collective_compute(
                            "ReduceScatter", ADD, ins=[foT_cc[c][:]],
                            outs=[fo_rs[c][:]], replica_groups=RG)
                        if c == NCH - 1:
                            residual(c)

    nc.compile()
    return nc


def _prep_inputs(x, freqs_cos, freqs_sin, mask, attn_norm_w, wq, wk, wv, wo,
                 ffn_norm_w, w1, w2, w3):
    """Host-side sharding + weight layout. Returns in_maps for 8 cores."""
    f32 = np.float32
    x2 = np.asarray(x, f32)[0]                     # [S, D]
    xT = np.ascontiguousarray(x2.T)                # [D, S]
    # SBUF-tile-ordered x: x_ch[c, p, kt, s] = xT[128*kt+p, 512*c+s]
    x_ch = np.ascontiguousarray(
        xT.astype(NP_CDT).reshape(DT, P, NCH, CW).transpose(2, 1, 0, 3))
    anw = np.asarray(attn_norm_w, f32)
    fnw = np.asarray(ffn_norm_w, f32)
    wq = np.asarray(wq, f32) * anw[None, :]
    wk = np.asarray(wk, f32) * anw[None, :]
    wv_e = np.asarray(wv, f32)
    wo = np.asarray(wo, f32)
    w1 = np.asarray(w1, f32) * fnw[None, :]
    w3 = np.asarray(w3, f32) * fnw[None, :]
    w2 = np.asarray(w2, f32)

    perm = np.concatenate([np.arange(0, HD, 2), np.arange(1, HD, 2)])

    cosT = np.ascontiguousarray(np.asarray(freqs_cos, f32).T)   # [64, S]
    sinT = np.ascontiguousarray(np.asarray(freqs_sin, f32).T)
    cos2 = np.concatenate([cosT, cosT], axis=0).astype(NP_CDT)  # [128, S]
    sinsg2 = np.concatenate([-sinT, sinT], axis=0).astype(NP_CDT)
    m = np.asarray(mask, f32)[0, 0]
    dmask = (np.ascontiguousarray(m[:P, :P].T) * f32(math.sqrt(HD))).astype(f32)

    def lhsT_tiles(wt, n_out_tiles, n_k_tiles):
        # wt: [K, Mout] -> [ot, p, kt, j] with [ot,p,kt,j] = wt[128*kt+p, 128*ot+j]
        a = wt.reshape(n_k_tiles, P, n_out_tiles, P)
        return np.ascontiguousarray(a.transpose(2, 1, 0, 3)).astype(NP_CDT)

    in_maps = []
    for r in range(CORES):
        ds = slice(DQ * r, DQ * (r + 1))
        wqT = wq[ds].T.copy()                      # [D, DQ]
        wkT = wk[ds].T.copy()
        for h in range(NHC):
            blk = slice(HD * h, HD * (h + 1))
            wqT[:, blk] = wqT[:, blk][:, perm]
            wkT[:, blk] = wkT[:, blk][:, perm]
        wqk = np.concatenate([lhsT_tiles(wqT, NHC, DT),
                              lhsT_tiles(wkT, NHC, DT)], axis=0)  # [8,P,DT,P]
        wvT = wv_e[ds].T.copy()                    # [D, DQ]
        # [P, DT, DQ]: 32KB contiguous per partition -> one efficient DMA
        w_v_l = np.ascontiguousarray(
            wvT.reshape(DT, P, DQ).transpose(1, 0, 2)).astype(NP_CDT)
        # wo ROW-sharded: contract own 512 attn dims, all 4096 out dims
        # w_o_l[p, ot, h, j] = wo[128*ot+j, 512*r + 128*h + p]
        a = np.ascontiguousarray(wo[:, ds].T)      # [512 d_own, 4096 o]
        w_o_l = np.ascontiguousarray(
            a.reshape(NHC, P, DT, P).transpose(1, 2, 0, 3)).astype(NP_CDT)
        fs = slice(FC * r, FC * (r + 1))
        w1s = np.zeros((FP, D), f32)
        w3s = np.zeros((FP, D), f32)
        w1s[:FC] = w1[fs]
        w3s[:FC] = w3[fs]
        w1_l = lhsT_tiles(np.ascontiguousarray(w1s.T), FT, DT)  # [FT, P, DT, P]
        w3_l = lhsT_tiles(np.ascontiguousarray(w3s.T), FT, DT)
        w2s = np.zeros((FP, D), f32)
        w2s[:FC] = w2[:, fs].T                     # [FP, D] (rows = f)
        w2_l = lhsT_tiles(w2s, 32, FT)             # [32, P, FT, P]

        in_maps.append({
            "xT_s": np.ascontiguousarray(xT[ds]),
            "x_ch": x_ch,
            "w_qk": wqk,
            "w_v": w_v_l,
            "w_o": w_o_l,
            "w_1": w1_l,
            "w_3": w3_l,
            "w_2": w2_l,
            "cos2": cos2,
            "sinsg2": sinsg2,
            "dmask": dmask,
        })
    return in_maps


def kernel(x, freqs_cos, freqs_sin, mask, attn_norm_w, wq, wk, wv, wo,
           ffn_norm_w, w1, w2, w3, _trace=False):
    global _COMPILED
    if _COMPILED is None:
        _COMPILED = _build()
    nc = _COMPILED
    in_maps = _prep_inputs(x, freqs_cos, freqs_sin, mask, attn_norm_w,
                           wq, wk, wv, wo, ffn_norm_w, w1, w2, w3)
    res = run_bass_kernel_spmd(nc, in_maps, list(range(CORES)), trace=_trace)
    kernel.last_result = res
    outT = np.concatenate([res.results[r]["outT_s"] for r in range(CORES)],
                          axis=0)                  # [D, S]
    return np.ascontiguousarray(outT.T)[None].astype(np.float32)


# revision 35
# speedup vs baseline: 1.0036x; 1.0036x over previous
"""Llama-style transformer block on 8 TRN2 NeuronCores.

v8: skew-immune design.  Cross-core launch skew (~25-70us) makes any
engine-FIFO instruction that waits on a collective a head-of-line hazard
(the tile scheduler hoists aggressively and does not model peer skew).
So v8 keeps ONLY the unavoidable big collectives (RS of wo partials, AG
of h, RS of FFN partials) and computes everything else locally:
  - x RMS stats: full sum-of-squares from the replicated x_ch tiles via
    ones-matmuls (x is replicated on every core anyway).  No AllReduce.
  - FFN RMS stats: from the gathered hn tiles via ones-matmuls, computed
    at each FFN chunk start (prefetched a chunk ahead).  No AllReduce.
  - h_block / residual are single fat DMAs + one wide DVE op, minimizing
    the number of FIFO slots that can block on an RS result.
  - wo ROW-sharded (no attnT AllGather); attention rowsums staggered one
    ktile behind scores; AV after rowsum chain so recip hides under it.
Program: A0..A3 | B0 B1 wo0 B2 wo1 B3 wo2 h0 wo3 h1 |
         ffn0[hn,stats,scale,ft,h2,h3,w2,RSf0] ffn1[...,res0,...] ...
"""

import math

import ml_dtypes
import numpy as np

import concourse.bass as bass
import concourse.mybir as mybir
import concourse.tile as tile
from concourse import bacc
from concourse.bass_utils import run_bass_kernel_spmd

S = 2048
D = 4096
HD = 128
NH = 32
F = 11008
CORES = 8
NHC = NH // CORES          # heads per core = 4
DQ = NHC * HD              # q/k/v dims per core = 512
FC = F // CORES            # ffn dims per core = 1376
FT = 11                    # padded f-tiles per core
FP = FT * 128
EPS = 1e-5
P = 128
NCH = 4                    # 512-token chunks
CW = S // NCH              # chunk width = 512
DT = D // P                # d tiles = 32
ST = S // P                # s tiles = 16

CDT = mybir.dt.bfloat16
NP_CDT = ml_dtypes.bfloat16

_COMPILED = None


def _build():
    nc = bacc.Bacc("TRN2", target_bir_lowering=False, debug=False,
                   num_devices=CORES)
    f32 = mybir.dt.float32

    # ---- kernel I/O ----
    xT_s = nc.declare_dram_parameter("xT_s", [DQ, S], f32, isOutput=False)
    x_ch = nc.declare_dram_parameter("x_ch", [NCH, P, DT, CW], CDT,
                                     isOutput=False)
    w_qk = nc.declare_dram_parameter("w_qk", [8, P, DT, P], CDT, isOutput=False)
    w_v = nc.declare_dram_parameter("w_v", [P, DT, DQ], CDT, isOutput=False)
    w_o = nc.declare_dram_parameter("w_o", [P, DT, NHC, P], CDT, isOutput=False)
    w_1 = nc.declare_dram_parameter("w_1", [FT, P, DT, P], CDT, isOutput=False)
    w_3 = nc.declare_dram_parameter("w_3", [FT, P, DT, P], CDT, isOutput=False)
    w_2 = nc.declare_dram_parameter("w_2", [32, P, FT, P], CDT, isOutput=False)
    cos2 = nc.declare_dram_parameter("cos2", [P, S], CDT, isOutput=False)
    sinsg2 = nc.declare_dram_parameter("sinsg2", [P, S], CDT, isOutput=False)
    dmask = nc.declare_dram_parameter("dmask", [P, P], f32, isOutput=False)
    outT_s = nc.declare_dram_parameter("outT_s", [DQ, S], f32, isOutput=True)

    # ---- internal DRAM ----
    s1row = nc.dram_tensor("s1row", [1, S], f32)
    oT_cc = [nc.dram_tensor(f"oT_cc{c}", [D, CW], CDT) for c in range(NCH)]
    o_rs = [nc.dram_tensor(f"o_rs{c}", [DQ, CW], CDT) for c in range(NCH)]
    h_cc = [nc.dram_tensor(f"h_cc{c}", [DQ, CW], CDT) for c in range(NCH)]
    hT_ag = [nc.dram_tensor(f"hT_ag{c}", [D, CW], CDT, addr_space="Shared")
             for c in range(NCH)]
    foT_cc = [nc.dram_tensor(f"foT_cc{c}", [D, CW], CDT) for c in range(NCH)]
    fo_rs = [nc.dram_tensor(f"fo_rs{c}", [DQ, CW], CDT) for c in range(NCH)]

    RG = [list(range(CORES))]
    ADD = mybir.AluOpType.add
    BYP = mybir.AluOpType.bypass
    EXP = mybir.ActivationFunctionType.Exp
    SQRT = mybir.ActivationFunctionType.Sqrt
    SILU = mybir.ActivationFunctionType.Silu
    ISQ = 1.0 / math.sqrt(HD)

    def ch(c):
        return slice(CW * c, CW * (c + 1))

    with tile.TileContext(nc) as tc:
        with (
            tc.tile_pool(name="persist", bufs=1) as persist,
        ):
            ones = persist.tile([P, 1], CDT)
            nc.vector.memset(ones[:], 1.0)
            eps_sb = persist.tile([P, 1], f32)
            nc.vector.memset(eps_sb[:], EPS)
            dmask_sb = persist.tile([P, P], f32)
            nc.gpsimd.dma_start(out=dmask_sb[:], in_=dmask[:])
            hTb = persist.tile([P, NHC, S], CDT)
            s1tok = persist.tile([P, ST], f32)

            with tc.tile_pool(name="qkvsb", bufs=1) as qkvsb:
                qts = [qkvsb.tile([P, S], CDT, tag=f"qt{h}", name=f"qt{h}")
                       for h in range(NHC)]
                kts = [qkvsb.tile([P, S], CDT, tag=f"kt{h}", name=f"kt{h}")
                       for h in range(NHC)]
                v_sb = qkvsb.tile([P, ST, DQ], CDT)

                # ======== stage A: local stats + Q/K/V (+RoPE) ========
                with (
                    tc.tile_pool(name="tbl", bufs=1) as tbl,
                    tc.tile_pool(name="xst1", bufs=2) as xst1,
                    tc.tile_pool(name="xst", bufs=2) as xst,
                    tc.tile_pool(name="stAx", bufs=5) as stAx,
                    tc.tile_pool(name="stAw", bufs=2) as stAw,
                    tc.tile_pool(name="wvp", bufs=1) as wvp,
                    tc.tile_pool(name="rope", bufs=2) as rope,
                    tc.tile_pool(name="ps_qkv", bufs=3, space="PSUM") as ps_qkv,
                    tc.tile_pool(name="ps_v", bufs=1, space="PSUM") as ps_v,
                    tc.tile_pool(name="ps_xst", bufs=1, space="PSUM") as ps_xst,
                ):
                    cos_raw = tbl.tile([P, S], CDT, tag="cosr")
                    sin_raw = tbl.tile([P, S], CDT, tag="sinr")
                    nc.gpsimd.dma_start(out=cos_raw[:], in_=cos2[:])
                    nc.gpsimd.dma_start(out=sin_raw[:], in_=sinsg2[:])
                    wv_sb = wvp.tile([P, DT, DQ], CDT)
                    nc.scalar.dma_start(out=wv_sb[:], in_=w_v[:])

                    for c in range(NCH):
                      with nc.named_scope(f"qkv_c{c}"):
                        xq = [stAx.tile([P, 8, CW], CDT, tag="xq",
                                        name=f"xq{j}_{c}")
                              for j in range(4)]
                        for j in range(4):
                            nc.gpsimd.dma_start(
                                out=xq[j][:],
                                in_=x_ch[c][:, 8 * j:8 * (j + 1), :])

                        def xkt(kt):
                            return xq[kt // 8][:, kt % 8, :]

                        # --- local RMS stats: full ssq from replicated x ---
                        pst = ps_xst.tile([1, CW], f32, tag="pst")
                        for j in range(4):
                            sqx = xst1.tile([P, 8, CW], CDT, tag="sqx",
                                            name=f"sqx{j}_{c}")
                            nc.vector.tensor_mul(sqx[:], xq[j][:], xq[j][:])
                            for kk in range(8):
                                nc.tensor.matmul(
                                    pst[:], ones[:], sqx[:, kk, :],
                                    start=(j == 0 and kk == 0),
                                    stop=(j == 3 and kk == 7))
                        row = xst.tile([1, CW], f32, tag="xrow",
                                       name=f"xrow{c}")
                        nc.scalar.activation(out=row[:], in_=pst[:],
                                             func=SQRT, bias=eps_sb[0:1],
                                             scale=1.0 / D)
                        nc.vector.reciprocal(out=row[:], in_=row[:])
                        # token-major copy for the V scale
                        nc.gpsimd.dma_start(out=s1row[0:1, ch(c)], in_=row[:])
                        nc.gpsimd.dma_start(
                            out=s1tok[:, 4 * c:4 * c + 4],
                            in_=s1row[0:1, ch(c)].rearrange(
                                "o (j p) -> p (o j)", p=P))
                        s1b = xst.tile([1, CW], CDT, tag="s1b", name=f"s1b{c}")
                        nc.vector.tensor_copy(out=s1b[:], in_=row[:])
                        s1rep = xst.tile([P, CW], CDT, tag="s1rep",
                                         name=f"s1rep{c}")
                        nc.gpsimd.partition_broadcast(s1rep[:], s1b[:])
                        cs_t = rope.tile([P, CW], CDT, tag="cs", name=f"cs{c}")
                        sn_t = rope.tile([P, CW], CDT, tag="sn", name=f"sn{c}")
                        nc.vector.tensor_mul(cs_t[:], cos_raw[:, ch(c)],
                                             s1rep[:])
                        nc.vector.tensor_mul(sn_t[:], sin_raw[:, ch(c)],
                                             s1rep[:])

                        # --- Q and K projections + RoPE ---
                        for ot in range(8):
                            wt = stAw.tile([P, DT, P], CDT, tag="wqk")
                            if ot % 2 == 0:
                                nc.scalar.dma_start(out=wt[:], in_=w_qk[ot])
                            else:
                                nc.sync.dma_start(out=wt[:], in_=w_qk[ot])
                            pt = ps_qkv.tile([P, CW], f32, tag="pqk")
                            for kt in range(DT):
                                nc.tensor.matmul(pt[:], wt[:, kt], xkt(kt),
                                                 start=(kt == 0),
                                                 stop=(kt == DT - 1))
                            swp = rope.tile([P, CW], f32, tag="swp")
                            nc.vector.tensor_copy(swp[0:64, :], pt[64:128, :])
                            nc.vector.tensor_copy(swp[64:128, :], pt[0:64, :])
                            t1 = rope.tile([P, CW], f32, tag="t1")
                            nc.vector.tensor_mul(t1[:], pt[:], cs_t[:])
                            nc.vector.tensor_mul(swp[:], swp[:], sn_t[:])
                            dst = qts[ot % 4] if ot < 4 else kts[ot % 4]
                            nc.vector.tensor_add(dst[:, ch(c)], t1[:], swp[:])

                        # --- V: 4 token-tiles of this chunk ---
                        pts = [ps_v.tile([P, DQ], f32, tag=f"pv{i}",
                                         name=f"pv{i}") for i in range(4)]
                        for kt in range(DT):
                            for i in range(4):
                                tok = slice(P * i, P * (i + 1))
                                nc.tensor.matmul(
                                    pts[i][:], xkt(kt)[:, tok],
                                    wv_sb[:, kt, :],
                                    start=(kt == 0), stop=(kt == DT - 1))
                        for i in range(4):
                            st = 4 * c + i
                            nc.vector.tensor_scalar_mul(
                                out=v_sb[:, st, :], in0=pts[i][:],
                                scalar1=s1tok[:, st:st + 1])

                H_TS = [0.66, 0.71, 0.77, 0.88]   # est. RS(o_c)-done, ms

                def h_block(c, hst, xtp):
                    # tile_wait_until keeps the scheduler from hoisting this
                    # RS-dependent block into an early FIFO slot (it would
                    # head-of-line block the queue until the RS lands).
                    with nc.named_scope(f"h_c{c}"), \
                         tc.tile_wait_until(H_TS[c]):
                        osb4 = hst.tile([P, NHC, CW], CDT, tag="osb")
                        nc.gpsimd.dma_start(
                            out=osb4[:],
                            in_=o_rs[c][:].rearrange("(i p) s -> p i s", p=P))
                        xt4 = xtp.tile([P, NHC, CW], f32, tag="xt")
                        nc.sync.dma_start(
                            out=xt4[:],
                            in_=xT_s[:, ch(c)].rearrange("(i p) s -> p i s",
                                                         p=P))
                        nc.vector.tensor_add(hTb[:, :, ch(c)], xt4[:],
                                             osb4[:])
                        nc.gpsimd.dma_start(
                            out=h_cc[c][:].rearrange("(p k) s -> p k s", p=P),
                            in_=hTb[:, :, ch(c)])
                        nc.gpsimd.collective_compute(
                            "AllGather", BYP, ins=[h_cc[c][:]],
                            outs=[hT_ag[c][:]], replica_groups=RG)

                # ======== stage B: attention + row-sharded wo ========
                with (
                    tc.tile_pool(name="stB", bufs=4) as stB,
                    tc.tile_pool(name="exps", bufs=18) as exps,
                    tc.tile_pool(name="attp", bufs=8) as attp,
                    tc.tile_pool(name="woW", bufs=1) as woW,
                    tc.tile_pool(name="hstB", bufs=2) as hstB,
                    tc.tile_pool(name="xtpB", bufs=2) as xtpB,
                    tc.tile_pool(name="ps_sc", bufs=3, space="PSUM") as ps_sc,
                    tc.tile_pool(name="ps_av", bufs=2, space="PSUM") as ps_av,
                    tc.tile_pool(name="ps_sm", bufs=1, space="PSUM") as ps_sm,
                    tc.tile_pool(name="ps_wo", bufs=2, space="PSUM") as ps_wo,
                ):
                    wo_sb = woW.tile([P, DT, NHC, P], CDT)
                    nc.sync.dma_start(out=wo_sb[:], in_=w_o[:])

                    at_ts = {}

                    def attn_chunk(qc):
                      with nc.named_scope(f"attn_c{qc}"):
                        nkt = 4 * qc + 4
                        for hh in range(NHC):
                            qt, kt_t = qts[hh], kts[hh]
                            smp = ps_sm.tile([1, CW], f32, tag="smp")
                            ets = []
                            for ktile in range(nkt):
                                diag = ktile >= 4 * qc
                                col0 = P * (ktile - 4 * qc) if diag else 0
                                scp = ps_sc.tile([P, CW], f32, tag="scp")
                                nc.tensor.matmul(
                                    scp[:, col0:],
                                    kt_t[:, P * ktile:P * (ktile + 1)],
                                    qt[:, CW * qc + col0:CW * (qc + 1)],
                                    start=True, stop=True)
                                if diag:
                                    nc.vector.tensor_add(
                                        scp[:, col0:col0 + P],
                                        scp[:, col0:col0 + P], dmask_sb[:])
                                et = exps.tile([P, CW], CDT, tag="et")
                                nc.scalar.activation(out=et[:, col0:],
                                                     in_=scp[:, col0:],
                                                     func=EXP, scale=ISQ)
                                ets.append((et, col0))
                                if ktile > 0:
                                    pe, pc0 = ets[ktile - 1]
                                    nc.tensor.matmul(smp[:, pc0:], ones[:],
                                                     pe[:, pc0:],
                                                     start=(ktile == 1),
                                                     stop=False)
                            pe, pc0 = ets[nkt - 1]
                            nc.tensor.matmul(smp[:, pc0:], ones[:],
                                             pe[:, pc0:], start=(nkt == 1),
                                             stop=True)
                            rec = stB.tile([1, CW], f32, tag="rec")
                            nc.vector.reciprocal(out=rec[:], in_=smp[:])
                            avp = ps_av.tile([P, CW], f32, tag="avp")
                            for ktile in range(nkt):
                                et, col0 = ets[ktile]
                                nc.tensor.matmul(
                                    avp[:, col0:],
                                    v_sb[:, ktile, P * hh:P * (hh + 1)],
                                    et[:, col0:], start=(ktile == 0),
                                    stop=(ktile == nkt - 1))
                            rrep = stB.tile([P, CW], f32, tag="rrep")
                            nc.gpsimd.partition_broadcast(rrep[:], rec[:])
                            att = attp.tile([P, CW], CDT, tag="att")
                            nc.vector.tensor_mul(att[:], avp[:], rrep[:])
                            at_ts[(qc, hh)] = att

                    def wo_mm(c):
                      with nc.named_scope(f"wo_c{c}"):
                        for ot in range(DT):
                            pt = ps_wo.tile([P, CW], f32, tag="pwo")
                            for kt in range(NHC):
                                nc.tensor.matmul(pt[:], wo_sb[:, ot, kt, :],
                                                 at_ts[(c, kt)][:],
                                                 start=(kt == 0),
                                                 stop=(kt == NHC - 1))
                            og = stB.tile([P, CW], CDT, tag="og")
                            if c == 3:
                                # exp traffic is over; free the DVE for the
                                # FFN-c0 warmup chain
                                nc.scalar.copy(out=og[:], in_=pt[:])
                            else:
                                nc.vector.tensor_copy(out=og[:], in_=pt[:])
                            nc.sync.dma_start(
                                out=oT_cc[c][P * ot:P * (ot + 1), :],
                                in_=og[:])
                        for kt in range(NHC):
                            del at_ts[(c, kt)]
                        nc.gpsimd.collective_compute(
                            "ReduceScatter", ADD, ins=[oT_cc[c][:]],
                            outs=[o_rs[c][:]], replica_groups=RG)

                    attn_chunk(0)
                    attn_chunk(1)
                    wo_mm(0)
                    attn_chunk(2)
                    wo_mm(1)
                    attn_chunk(3)
                    wo_mm(2)
                    h_block(0, hstB, xtpB)
                    wo_mm(3)
                    h_block(1, hstB, xtpB)

            # ====== stage D: FFN (local stats) + chunked RS + residual ====
            with (
                tc.tile_pool(name="stDh", bufs=2) as stDh,
                tc.tile_pool(name="stDs", bufs=2) as stDs,
                tc.tile_pool(name="stDw", bufs=2) as stDw,
                tc.tile_pool(name="stDw2", bufs=3) as stDw2,
                tc.tile_pool(name="stDg", bufs=2) as stDg,
                tc.tile_pool(name="stDt", bufs=3) as stDt,
                tc.tile_pool(name="hstD", bufs=2) as hstD,
                tc.tile_pool(name="xtpD", bufs=2) as xtpD,
                tc.tile_pool(name="ps_f1", bufs=2, space="PSUM") as ps_f1,
                tc.tile_pool(name="ps_f3", bufs=2, space="PSUM") as ps_f3,
                tc.tile_pool(name="ps_w2", bufs=2, space="PSUM") as ps_w2,
                tc.tile_pool(name="ps_hst", bufs=1, space="PSUM") as ps_hst,
            ):

                RES_TS = [1.38, 1.68, 1.98, 2.26]  # est. RS(f_c)-done, ms

                def residual(c):
                    # RS(f_c)-dependent: keep out of early FIFO slots.
                    with nc.named_scope(f"res_c{c}"), \
                         tc.tile_wait_until(RES_TS[c]):
                        osb4 = hstD.tile([P, NHC, CW], CDT, tag="osb")
                        nc.gpsimd.dma_start(
                            out=osb4[:],
                            in_=fo_rs[c][:].rearrange("(i p) s -> p i s",
                                                      p=P))
                        res4 = hstD.tile([P, NHC, CW], f32, tag="res")
                        nc.vector.tensor_add(res4[:], hTb[:, :, ch(c)],
                                             osb4[:])
                        nc.sync.dma_start(
                            out=outT_s[:, ch(c)].rearrange(
                                "(i p) s -> p i s", p=P),
                            in_=res4[:])

                HN_TS = [0.81, 0.94, 0.99, 1.04]   # est. AG(h_c)-done, ms

                for c in range(NCH):
                    with nc.named_scope(f"ffn_c{c}"):
                        hn_sb = stDh.tile([P, DT, CW], CDT, tag="hn")
                        # (k s) merged: each (p, r) is a 4KB contiguous run;
                        # split over all three DMA queues for bandwidth
                        hgv = hT_ag[c][:].rearrange(
                            "(r p k) s -> p r (k s)", p=P, k=NHC)
                        with tc.tile_wait_until(HN_TS[c]):
                            nc.gpsimd.dma_start(out=hn_sb[:, 0:16, :],
                                                in_=hgv[:, 0:4, :])
                            nc.scalar.dma_start(out=hn_sb[:, 16:32, :],
                                                in_=hgv[:, 4:8, :])
                        # local FFN RMS stats from the gathered h
                        psh = ps_hst.tile([1, CW], f32, tag="psh")
                        for kt in range(DT):
                            sq = stDs.tile([P, CW], CDT, tag="sq")
                            nc.vector.tensor_mul(sq[:], hn_sb[:, kt, :],
                                                 hn_sb[:, kt, :])
                            nc.tensor.matmul(psh[:], ones[:], sq[:],
                                             start=(kt == 0),
                                             stop=(kt == DT - 1))
                        rh = stDs.tile([1, CW], f32, tag="hrow")
                        nc.scalar.activation(out=rh[:], in_=psh[:], func=SQRT,
                                             bias=eps_sb[0:1], scale=1.0 / D)
                        nc.vector.reciprocal(out=rh[:], in_=rh[:])
                        s2b = stDs.tile([1, CW], CDT, tag="s2b")
                        nc.vector.tensor_copy(out=s2b[:], in_=rh[:])
                        s2l = stDs.tile([P, CW], CDT, tag="s2l")
                        nc.gpsimd.partition_broadcast(s2l[:], s2b[:])
                        for kt in range(DT):
                            nc.vector.tensor_mul(hn_sb[:, kt, :],
                                                 hn_sb[:, kt, :], s2l[:])
                        g_sb = stDg.tile([P, FT, CW], CDT, tag="g")
                        for ft in range(FT):
                            w1t = stDw.tile([P, DT, P], CDT, tag="w1")
                            w3t = stDw.tile([P, DT, P], CDT, tag="w3")
                            nc.scalar.dma_start(out=w1t[:], in_=w_1[ft])
                            nc.sync.dma_start(out=w3t[:], in_=w_3[ft])
                            p1 = ps_f1.tile([P, CW], f32, tag="p1")
                            p3 = ps_f3.tile([P, CW], f32, tag="p3")
                            for kt in range(DT):
                                nc.tensor.matmul(p1[:], w1t[:, kt],
                                                 hn_sb[:, kt, :],
                                                 start=(kt == 0),
                                                 stop=(kt == DT - 1))
                            for kt in range(DT):
                                nc.tensor.matmul(p3[:], w3t[:, kt],
                                                 hn_sb[:, kt, :],
                                                 start=(kt == 0),
                                                 stop=(kt == DT - 1))
                            tsi = stDt.tile([P, CW], CDT, tag="tsi")
                            nc.scalar.activation(out=tsi[:], in_=p1[:],
                                                 func=SILU)
                            nc.vector.tensor_mul(g_sb[:, ft, :], tsi[:],
                                                 p3[:])
                        if c == 0:
                            h_block(2, hstD, xtpD)
                            h_block(3, hstD, xtpD)
                        else:
                            residual(c - 1)
                        for ot in range(32):
                            w2t = stDw2.tile([P, FT, P], CDT, tag="w2")
                            if ot % 2 == 0:
                                nc.scalar.dma_start(out=w2t[:], in_=w_2[ot])
                            else:
                                nc.sync.dma_start(out=w2t[:], in_=w_2[ot])
                            pt = ps_w2.tile([P, CW], f32, tag="pw2")
                            for ft in range(FT):
                                nc.tensor.matmul(pt[:], w2t[:, ft],
                                                 g_sb[:, ft, :],
                                                 start=(ft == 0),
                                                 stop=(ft == FT - 1))
                            og = stDt.tile([P, CW], CDT, tag="og")
                            if ot % 2 == 0:
                                nc.vector.tensor_copy(out=og[:], in_=pt[:])
                            else:
                                nc.scalar.copy(out=og[:], in_=pt[:])
                            nc.sync.dma_start(
                                out=foT_cc[c][P * ot:P * (ot + 1), :],
                                in_=og[:])
                        nc.gpsimd.collective_compute(
                            "ReduceScatter", ADD, ins=[foT_cc[c][:]],
                            outs=[fo_rs[c][:]], replica_groups=RG)
                        if c == NCH - 1:
                            residual(c)

    nc.compile()
    return nc


def _prep_inputs(x, freqs_cos, freqs_sin, mask, attn_norm_w, wq, wk, wv, wo,
                 ffn_norm_w, w1, w2, w3):
    """Host-side sharding + weight layout. Returns in_maps for 8 cores."""
    f32 = np.float32
    x2 = np.asarray(x, f32)[0]                     # [S, D]
    xT = np.ascontiguousarray(x2.T)                # [D, S]
    # SBUF-tile-ordered x: x_ch[c, p, kt, s] = xT[128*kt+p, 512*c+s]
    x_ch = np.ascontiguousarray(
        xT.astype(NP_CDT).reshape(DT, P, NCH, CW).transpose(2, 1, 0, 3))
    anw = np.asarray(attn_norm_w, f32)
    fnw = np.asarray(ffn_norm_w, f32)
    wq = np.asarray(wq, f32) * anw[None, :]
    wk = np.asarray(wk, f32) * anw[None, :]
    wv_e = np.asarray(wv, f32)
    wo = np.asarray(wo, f32)
    w1 = np.asarray(w1, f32) * fnw[None, :]
    w3 = np.asarray(w3, f32) * fnw[None, :]
    w2 = np.asarray(w2, f32)

    perm = np.concatenate([np.arange(0, HD, 2), np.arange(1, HD, 2)])

    cosT = np.ascontiguousarray(np.asarray(freqs_cos, f32).T)   # [64, S]
    sinT = np.ascontiguousarray(np.asarray(freqs_sin, f32).T)
    cos2 = np.concatenate([cosT, cosT], axis=0).astype(NP_CDT)  # [128, S]
    sinsg2 = np.concatenate([-sinT, sinT], axis=0).astype(NP_CDT)
    m = np.asarray(mask, f32)[0, 0]
    dmask = (np.ascontiguousarray(m[:P, :P].T) * f32(math.sqrt(HD))).astype(f32)

    def lhsT_tiles(wt, n_out_tiles, n_k_tiles):
        # wt: [K, Mout] -> [ot, p, kt, j] with [ot,p,kt,j] = wt[128*kt+p, 128*ot+j]
        a = wt.reshape(n_k_tiles, P, n_out_tiles, P)
        return np.ascontiguousarray(a.transpose(2, 1, 0, 3)).astype(NP_CDT)

    in_maps = []
    for r in range(CORES):
        ds = slice(DQ * r, DQ * (r + 1))
        wqT = wq[ds].T.copy()                      # [D, DQ]
        wkT = wk[ds].T.copy()
        for h in range(NHC):
            blk = slice(HD * h, HD * (h + 1))
            wqT[:, blk] = wqT[:, blk][:, perm]
            wkT[:, blk] = wkT[:, blk][:, perm]
        wqk = np.concatenate([lhsT_tiles(wqT, NHC, DT),
                              lhsT_tiles(wkT, NHC, DT)], axis=0)  # [8,P,DT,P]
        wvT = wv_e[ds].T.copy()                    # [D, DQ]
        # [P, DT, DQ]: 32KB contiguous per partition -> one efficient DMA
        w_v_l = np.ascontiguousarray(
            wvT.reshape(DT, P, DQ).transpose(1, 0, 2)).astype(NP_CDT)
        # wo ROW-sharded: contract own 512 attn dims, all 4096 out dims
        # w_o_l[p, ot, h, j] = wo[128*ot+j, 512*r + 128*h + p]
        a = np.ascontiguousarray(wo[:, ds].T)      # [512 d_own, 4096 o]
        w_o_l = np.ascontiguousarray(
            a.reshape(NHC, P, DT, P).transpose(1, 2, 0, 3)).astype(NP_CDT)
        fs = slice(FC * r, FC * (r + 1))
        w1s = np.zeros((FP, D), f32)
        w3s = np.zeros((FP, D), f32)
        w1s[:FC] = w1[fs]
        w3s[:FC] = w3[fs]
        w1_l = lhsT_tiles(np.ascontiguousarray(w1s.T), FT, DT)  # [FT, P, DT, P]
        w3_l = lhsT_tiles(np.ascontiguousarray(w3s.T), FT, DT)
        w2s = np.zeros((FP, D), f32)
        w2s[:FC] = w2[:, fs].T                     # [FP, D] (rows = f)
        w2_l = lhsT_tiles(w2s, 32, FT)             # [32, P, FT, P]

        in_maps.append({
            "xT_s": np.ascontiguousarray(xT[ds]),
            "x_ch": x_ch,
            "w_qk": wqk,
            "w_v": w_v_l,
            "w_o": w_o_l,
            "w_1": w1_l,
            "w_3": w3_l,
            "w_2": w2_l,
            "cos2": cos2,
            "sinsg2": sinsg2,
            "dmask": dmask,
        })
    return in_maps


def kernel(x, freqs_cos, freqs_sin, mask, attn_norm_w, wq, wk, wv, wo,
           ffn_norm_w, w1, w2, w3, _trace=False):
    global _COMPILED
    if _COMPILED is None:
        _COMPILED = _build()
    nc = _COMPILED
    in_maps = _prep_inputs(x, freqs_cos, freqs_sin, mask, attn_norm_w,
                           wq, wk, wv, wo, ffn_norm_w, w1, w2, w3)
    res = run_bass_kernel_spmd(nc, in_maps, list(range(CORES)), trace=_trace)
    kernel.last_result = res
    outT = np.concatenate([res.results[r]["outT_s"] for r in range(CORES)],
                          axis=0)                  # [D, S]
    return np.ascontiguousarray(outT.T)[None].astype(np.float32)


# revision 39
# speedup vs baseline: 1.0233x; 1.0196x over previous
"""Llama-style transformer block on 8 TRN2 NeuronCores.

v8: skew-immune design.  Cross-core launch skew (~25-70us) makes any
engine-FIFO instruction that waits on a collective a head-of-line hazard
(the tile scheduler hoists aggressively and does not model peer skew).
So v8 keeps ONLY the unavoidable big collectives (RS of wo partials, AG
of h, RS of FFN partials) and computes everything else locally:
  - x RMS stats: full sum-of-squares from the replicated x_ch tiles via
    ones-matmuls (x is replicated on every core anyway).  No AllReduce.
  - FFN RMS stats: from the gathered hn tiles via ones-matmuls, computed
    at each FFN chunk start (prefetched a chunk ahead).  No AllReduce.
  - h_block / residual are single fat DMAs + one wide DVE op, minimizing
    the number of FIFO slots that can block on an RS result.
  - wo ROW-sharded (no attnT AllGather); attention rowsums staggered one
    ktile behind scores; AV after rowsum chain so recip hides under it.
Program: A0..A3 | B0 B1 wo0 B2 wo1 B3 wo2 h0 wo3 h1 |
         ffn0[hn,stats,scale,ft,h2,h3,w2,RSf0] ffn1[...,res0,...] ...
"""

import math

import ml_dtypes
import numpy as np

import concourse.bass as bass
import concourse.mybir as mybir
import concourse.tile as tile
from concourse import bacc
from concourse.bass_utils import run_bass_kernel_spmd

S = 2048
D = 4096
HD = 128
NH = 32
F = 11008
CORES = 8
NHC = NH // CORES          # heads per core = 4
DQ = NHC * HD              # q/k/v dims per core = 512
FC = F // CORES            # ffn dims per core = 1376
FT = 11                    # padded f-tiles per core
FP = FT * 128
EPS = 1e-5
P = 128
NCH = 4                    # 512-token chunks
CW = S // NCH              # chunk width = 512
DT = D // P                # d tiles = 32
ST = S // P                # s tiles = 16

CDT = mybir.dt.bfloat16
NP_CDT = ml_dtypes.bfloat16

_COMPILED = None


def _build():
    nc = bacc.Bacc("TRN2", target_bir_lowering=False, debug=False,
                   num_devices=CORES)
    f32 = mybir.dt.float32

    # ---- kernel I/O ----
    xT_s = nc.declare_dram_parameter("xT_s", [DQ, S], f32, isOutput=False)
    x_ch = nc.declare_dram_parameter("x_ch", [NCH, P, DT, CW], CDT,
                                     isOutput=False)
    w_qk = nc.declare_dram_parameter("w_qk", [8, P, DT, P], CDT, isOutput=False)
    w_v = nc.declare_dram_parameter("w_v", [P, DT, DQ], CDT, isOutput=False)
    w_o = nc.declare_dram_parameter("w_o", [P, DT, NHC, P], CDT, isOutput=False)
    w_1 = nc.declare_dram_parameter("w_1", [FT, P, DT, P], CDT, isOutput=False)
    w_3 = nc.declare_dram_parameter("w_3", [FT, P, DT, P], CDT, isOutput=False)
    w_2 = nc.declare_dram_parameter("w_2", [32, P, FT, P], CDT, isOutput=False)
    cos2 = nc.declare_dram_parameter("cos2", [P, S], CDT, isOutput=False)
    sinsg2 = nc.declare_dram_parameter("sinsg2", [P, S], CDT, isOutput=False)
    dmask = nc.declare_dram_parameter("dmask", [P, P], f32, isOutput=False)
    outT_s = nc.declare_dram_parameter("outT_s", [DQ, S], f32, isOutput=True)

    # ---- internal DRAM ----
    s1row = nc.dram_tensor("s1row", [1, S], f32)
    bar_p = nc.dram_tensor("bar_p", [1, 16], f32)
    bar_a = nc.dram_tensor("bar_a", [1, 16], f32)
    oT_cc = [nc.dram_tensor(f"oT_cc{c}", [D, CW], CDT) for c in range(NCH)]
    o_rs = [nc.dram_tensor(f"o_rs{c}", [DQ, CW], CDT) for c in range(NCH)]
    h_cc = [nc.dram_tensor(f"h_cc{c}", [DQ, CW], CDT) for c in range(NCH)]
    hT_ag = [nc.dram_tensor(f"hT_ag{c}", [D, CW], CDT, addr_space="Shared")
             for c in range(NCH)]
    foT_cc = [nc.dram_tensor(f"foT_cc{c}", [D, CW], CDT) for c in range(NCH)]
    fo_rs = [nc.dram_tensor(f"fo_rs{c}", [DQ, CW], CDT) for c in range(NCH)]

    RG = [list(range(CORES))]
    ADD = mybir.AluOpType.add
    BYP = mybir.AluOpType.bypass
    EXP = mybir.ActivationFunctionType.Exp
    SQRT = mybir.ActivationFunctionType.Sqrt
    SILU = mybir.ActivationFunctionType.Silu
    ISQ = 1.0 / math.sqrt(HD)

    def ch(c):
        return slice(CW * c, CW * (c + 1))

    with tile.TileContext(nc) as tc:
        with (
            tc.tile_pool(name="persist", bufs=1) as persist,
        ):
            ones = persist.tile([P, 1], CDT)
            nc.vector.memset(ones[:], 1.0)
            eps_sb = persist.tile([P, 1], f32)
            nc.vector.memset(eps_sb[:], EPS)
            dmask_sb = persist.tile([P, P], f32)
            nc.gpsimd.dma_start(out=dmask_sb[:], in_=dmask[:])
            hTb = persist.tile([P, NHC, S], CDT)
            s1tok = persist.tile([P, ST], f32)

            with tc.tile_pool(name="qkvsb", bufs=1) as qkvsb:
                qts = [qkvsb.tile([P, S], CDT, tag=f"qt{h}", name=f"qt{h}")
                       for h in range(NHC)]
                kts = [qkvsb.tile([P, S], CDT, tag=f"kt{h}", name=f"kt{h}")
                       for h in range(NHC)]
                v_sb = qkvsb.tile([P, ST, DQ], CDT)

                # ======== stage A: local stats + Q/K/V (+RoPE) ========
                with (
                    tc.tile_pool(name="tbl", bufs=1) as tbl,
                    tc.tile_pool(name="xst1", bufs=2) as xst1,
                    tc.tile_pool(name="xst", bufs=2) as xst,
                    tc.tile_pool(name="stAx", bufs=5) as stAx,
                    tc.tile_pool(name="stAw", bufs=2) as stAw,
                    tc.tile_pool(name="wvp", bufs=1) as wvp,
                    tc.tile_pool(name="rope", bufs=2) as rope,
                    tc.tile_pool(name="ps_qkv", bufs=3, space="PSUM") as ps_qkv,
                    tc.tile_pool(name="ps_v", bufs=1, space="PSUM") as ps_v,
                    tc.tile_pool(name="ps_xst", bufs=1, space="PSUM") as ps_xst,
                ):
                    # dummy barrier: absorbs cross-core launch skew under
                    # stage A so the first real collective starts aligned
                    nc.gpsimd.collective_compute(
                        "AllReduce", ADD, ins=[bar_p[:]], outs=[bar_a[:]],
                        replica_groups=RG)
                    cos_raw = tbl.tile([P, S], CDT, tag="cosr")
                    sin_raw = tbl.tile([P, S], CDT, tag="sinr")
                    nc.gpsimd.dma_start(out=cos_raw[:], in_=cos2[:])
                    nc.gpsimd.dma_start(out=sin_raw[:], in_=sinsg2[:])
                    wv_sb = wvp.tile([P, DT, DQ], CDT)
                    nc.scalar.dma_start(out=wv_sb[:], in_=w_v[:])

                    for c in range(NCH):
                      with nc.named_scope(f"qkv_c{c}"):
                        xq = [stAx.tile([P, 8, CW], CDT, tag="xq",
                                        name=f"xq{j}_{c}")
                              for j in range(4)]
                        for j in range(4):
                            nc.gpsimd.dma_start(
                                out=xq[j][:],
                                in_=x_ch[c][:, 8 * j:8 * (j + 1), :])

                        def xkt(kt):
                            return xq[kt // 8][:, kt % 8, :]

                        # --- local RMS stats: full ssq from replicated x ---
                        pst = ps_xst.tile([1, CW], f32, tag="pst")
                        for j in range(4):
                            sqx = xst1.tile([P, 8, CW], CDT, tag="sqx",
                                            name=f"sqx{j}_{c}")
                            nc.vector.tensor_mul(sqx[:], xq[j][:], xq[j][:])
                            for kk in range(8):
                                nc.tensor.matmul(
                                    pst[:], ones[:], sqx[:, kk, :],
                                    start=(j == 0 and kk == 0),
                                    stop=(j == 3 and kk == 7))
                        row = xst.tile([1, CW], f32, tag="xrow",
                                       name=f"xrow{c}")
                        nc.scalar.activation(out=row[:], in_=pst[:],
                                             func=SQRT, bias=eps_sb[0:1],
                                             scale=1.0 / D)
                        nc.vector.reciprocal(out=row[:], in_=row[:])
                        # token-major copy for the V scale
                        nc.gpsimd.dma_start(out=s1row[0:1, ch(c)], in_=row[:])
                        nc.gpsimd.dma_start(
                            out=s1tok[:, 4 * c:4 * c + 4],
                            in_=s1row[0:1, ch(c)].rearrange(
                                "o (j p) -> p (o j)", p=P))
                        s1b = xst.tile([1, CW], CDT, tag="s1b", name=f"s1b{c}")
                        nc.vector.tensor_copy(out=s1b[:], in_=row[:])
                        s1rep = xst.tile([P, CW], CDT, tag="s1rep",
                                         name=f"s1rep{c}")
                        nc.gpsimd.partition_broadcast(s1rep[:], s1b[:])
                        cs_t = rope.tile([P, CW], CDT, tag="cs", name=f"cs{c}")
                        sn_t = rope.tile([P, CW], CDT, tag="sn", name=f"sn{c}")
                        nc.vector.tensor_mul(cs_t[:], cos_raw[:, ch(c)],
                                             s1rep[:])
                        nc.vector.tensor_mul(sn_t[:], sin_raw[:, ch(c)],
                                             s1rep[:])

                        # --- Q and K projections + RoPE ---
                        for ot in range(8):
                            wt = stAw.tile([P, DT, P], CDT, tag="wqk")
                            if ot % 2 == 0:
                                nc.scalar.dma_start(out=wt[:], in_=w_qk[ot])
                            else:
                                nc.sync.dma_start(out=wt[:], in_=w_qk[ot])
                            pt = ps_qkv.tile([P, CW], f32, tag="pqk")
                            for kt in range(DT):
                                nc.tensor.matmul(pt[:], wt[:, kt], xkt(kt),
                                                 start=(kt == 0),
                                                 stop=(kt == DT - 1))
                            swp = rope.tile([P, CW], f32, tag="swp")
                            nc.vector.tensor_copy(swp[0:64, :], pt[64:128, :])
                            nc.vector.tensor_copy(swp[64:128, :], pt[0:64, :])
                            t1 = rope.tile([P, CW], f32, tag="t1")
                            nc.vector.tensor_mul(t1[:], pt[:], cs_t[:])
                            nc.vector.tensor_mul(swp[:], swp[:], sn_t[:])
                            dst = qts[ot % 4] if ot < 4 else kts[ot % 4]
                            nc.vector.tensor_add(dst[:, ch(c)], t1[:], swp[:])

                        # --- V: 4 token-tiles of this chunk ---
                        pts = [ps_v.tile([P, DQ], f32, tag=f"pv{i}",
                                         name=f"pv{i}") for i in range(4)]
                        for kt in range(DT):
                            for i in range(4):
                                tok = slice(P * i, P * (i + 1))
                                nc.tensor.matmul(
                                    pts[i][:], xkt(kt)[:, tok],
                                    wv_sb[:, kt, :],
                                    start=(kt == 0), stop=(kt == DT - 1))
                        for i in range(4):
                            st = 4 * c + i
                            nc.vector.tensor_scalar_mul(
                                out=v_sb[:, st, :], in0=pts[i][:],
                                scalar1=s1tok[:, st:st + 1])

                H_TS = [0.64, 0.70, 0.76, 0.86]   # est. RS(o_c)-done, ms

                def h_block(c, hst, xtp):
                    # tile_wait_until keeps the scheduler from hoisting this
                    # RS-dependent block into an early FIFO slot (it would
                    # head-of-line block the queue until the RS lands).
                    with nc.named_scope(f"h_c{c}"), \
                         tc.tile_wait_until(H_TS[c]):
                        osb4 = hst.tile([P, NHC, CW], CDT, tag="osb")
                        nc.gpsimd.dma_start(
                            out=osb4[:],
                            in_=o_rs[c][:].rearrange("(i p) s -> p i s", p=P))
                        xt4 = xtp.tile([P, NHC, CW], f32, tag="xt")
                        nc.sync.dma_start(
                            out=xt4[:],
                            in_=xT_s[:, ch(c)].rearrange("(i p) s -> p i s",
                                                         p=P))
                        nc.vector.tensor_add(hTb[:, :, ch(c)], xt4[:],
                                             osb4[:])
                        nc.gpsimd.dma_start(
                            out=h_cc[c][:].rearrange("(p k) s -> p k s", p=P),
                            in_=hTb[:, :, ch(c)])
                        nc.gpsimd.collective_compute(
                            "AllGather", BYP, ins=[h_cc[c][:]],
                            outs=[hT_ag[c][:]], replica_groups=RG)

                # ======== stage B: attention + row-sharded wo ========
                with (
                    tc.tile_pool(name="stB", bufs=4) as stB,
                    tc.tile_pool(name="exps", bufs=18) as exps,
                    tc.tile_pool(name="attp", bufs=8) as attp,
                    tc.tile_pool(name="woW", bufs=1) as woW,
                    tc.tile_pool(name="hstB", bufs=2) as hstB,
                    tc.tile_pool(name="xtpB", bufs=2) as xtpB,
                    tc.tile_pool(name="ps_sc", bufs=3, space="PSUM") as ps_sc,
                    tc.tile_pool(name="ps_av", bufs=2, space="PSUM") as ps_av,
                    tc.tile_pool(name="ps_sm", bufs=1, space="PSUM") as ps_sm,
                    tc.tile_pool(name="ps_wo", bufs=2, space="PSUM") as ps_wo,
                ):
                    wo_sb = woW.tile([P, DT, NHC, P], CDT)
                    nc.sync.dma_start(out=wo_sb[:], in_=w_o[:])

                    at_ts = {}

                    def attn_chunk(qc):
                      with nc.named_scope(f"attn_c{qc}"):
                        nkt = 4 * qc + 4
                        for hh in range(NHC):
                            qt, kt_t = qts[hh], kts[hh]
                            smp = ps_sm.tile([1, CW], f32, tag="smp")
                            ets = []
                            for ktile in range(nkt):
                                diag = ktile >= 4 * qc
                                col0 = P * (ktile - 4 * qc) if diag else 0
                                scp = ps_sc.tile([P, CW], f32, tag="scp")
                                nc.tensor.matmul(
                                    scp[:, col0:],
                                    kt_t[:, P * ktile:P * (ktile + 1)],
                                    qt[:, CW * qc + col0:CW * (qc + 1)],
                                    start=True, stop=True)
                                if diag:
                                    nc.vector.tensor_add(
                                        scp[:, col0:col0 + P],
                                        scp[:, col0:col0 + P], dmask_sb[:])
                                et = exps.tile([P, CW], CDT, tag="et")
                                nc.scalar.activation(out=et[:, col0:],
                                                     in_=scp[:, col0:],
                                                     func=EXP, scale=ISQ)
                                ets.append((et, col0))
                                if ktile > 0:
                                    pe, pc0 = ets[ktile - 1]
                                    nc.tensor.matmul(smp[:, pc0:], ones[:],
                                                     pe[:, pc0:],
                                                     start=(ktile == 1),
                                                     stop=False)
                            pe, pc0 = ets[nkt - 1]
                            nc.tensor.matmul(smp[:, pc0:], ones[:],
                                             pe[:, pc0:], start=(nkt == 1),
                                             stop=True)
                            rec = stB.tile([1, CW], f32, tag="rec")
                            nc.vector.reciprocal(out=rec[:], in_=smp[:])
                            avp = ps_av.tile([P, CW], f32, tag="avp")
                            for ktile in range(nkt):
                                et, col0 = ets[ktile]
                                nc.tensor.matmul(
                                    avp[:, col0:],
                                    v_sb[:, ktile, P * hh:P * (hh + 1)],
                                    et[:, col0:], start=(ktile == 0),
                                    stop=(ktile == nkt - 1))
                            rrep = stB.tile([P, CW], f32, tag="rrep")
                            nc.gpsimd.partition_broadcast(rrep[:], rec[:])
                            att = attp.tile([P, CW], CDT, tag="att")
                            nc.vector.tensor_mul(att[:], avp[:], rrep[:])
                            at_ts[(qc, hh)] = att

                    def wo_mm(c):
                      with nc.named_scope(f"wo_c{c}"):
                        for ot in range(DT):
                            pt = ps_wo.tile([P, CW], f32, tag="pwo")
                            for kt in range(NHC):
                                nc.tensor.matmul(pt[:], wo_sb[:, ot, kt, :],
                                                 at_ts[(c, kt)][:],
                                                 start=(kt == 0),
                                                 stop=(kt == NHC - 1))
                            og = stB.tile([P, CW], CDT, tag="og")
                            if c == 3:
                                # exp traffic is over; free the DVE for the
                                # FFN-c0 warmup chain
                                nc.scalar.copy(out=og[:], in_=pt[:])
                            else:
                                nc.vector.tensor_copy(out=og[:], in_=pt[:])
                            nc.sync.dma_start(
                                out=oT_cc[c][P * ot:P * (ot + 1), :],
                                in_=og[:])
                        for kt in range(NHC):
                            del at_ts[(c, kt)]
                        nc.gpsimd.collective_compute(
                            "ReduceScatter", ADD, ins=[oT_cc[c][:]],
                            outs=[o_rs[c][:]], replica_groups=RG)

                    attn_chunk(0)
                    attn_chunk(1)
                    wo_mm(0)
                    attn_chunk(2)
                    wo_mm(1)
                    attn_chunk(3)
                    wo_mm(2)
                    h_block(0, hstB, xtpB)
                    wo_mm(3)
                    h_block(1, hstB, xtpB)

            # ====== stage D: FFN (local stats) + chunked RS + residual ====
            with (
                tc.tile_pool(name="stDh", bufs=2) as stDh,
                tc.tile_pool(name="stDs", bufs=2) as stDs,
                tc.tile_pool(name="stDw", bufs=2) as stDw,
                tc.tile_pool(name="stDw2", bufs=3) as stDw2,
                tc.tile_pool(name="stDg", bufs=2) as stDg,
                tc.tile_pool(name="stDt", bufs=3) as stDt,
                tc.tile_pool(name="hstD", bufs=2) as hstD,
                tc.tile_pool(name="xtpD", bufs=2) as xtpD,
                tc.tile_pool(name="ps_f1", bufs=2, space="PSUM") as ps_f1,
                tc.tile_pool(name="ps_f3", bufs=2, space="PSUM") as ps_f3,
                tc.tile_pool(name="ps_w2", bufs=2, space="PSUM") as ps_w2,
                tc.tile_pool(name="ps_hst", bufs=1, space="PSUM") as ps_hst,
            ):

                RES_TS = [1.38, 1.68, 1.98, 2.26]  # est. RS(f_c)-done, ms

                def residual(c):
                    # RS(f_c)-dependent: keep out of early FIFO slots.
                    with nc.named_scope(f"res_c{c}"), \
                         tc.tile_wait_until(RES_TS[c]):
                        osb4 = hstD.tile([P, NHC, CW], CDT, tag="osb")
                        nc.gpsimd.dma_start(
                            out=osb4[:],
                            in_=fo_rs[c][:].rearrange("(i p) s -> p i s",
                                                      p=P))
                        res4 = hstD.tile([P, NHC, CW], f32, tag="res")
                        nc.vector.tensor_add(res4[:], hTb[:, :, ch(c)],
                                             osb4[:])
                        nc.sync.dma_start(
                            out=outT_s[:, ch(c)].rearrange(
                                "(i p) s -> p i s", p=P),
                            in_=res4[:])

                HN_TS = [0.72, 0.88, 0.94, 1.00]   # est. AG(h_c)-done, ms

                for c in range(NCH):
                    with nc.named_scope(f"ffn_c{c}"):
                        hn_sb = stDh.tile([P, DT, CW], CDT, tag="hn")
                        # (k s) merged: each (p, r) is a 4KB contiguous run;
                        # split over all three DMA queues for bandwidth
                        hgv = hT_ag[c][:].rearrange(
                            "(r p k) s -> p r (k s)", p=P, k=NHC)
                        with tc.tile_wait_until(HN_TS[c]):
                            nc.gpsimd.dma_start(out=hn_sb[:, 0:16, :],
                                                in_=hgv[:, 0:4, :])
                            nc.scalar.dma_start(out=hn_sb[:, 16:32, :],
                                                in_=hgv[:, 4:8, :])
                        # local FFN RMS stats from the gathered h
                        psh = ps_hst.tile([1, CW], f32, tag="psh")
                        for kt in range(DT):
                            sq = stDs.tile([P, CW], CDT, tag="sq")
                            nc.vector.tensor_mul(sq[:], hn_sb[:, kt, :],
                                                 hn_sb[:, kt, :])
                            nc.tensor.matmul(psh[:], ones[:], sq[:],
                                             start=(kt == 0),
                                             stop=(kt == DT - 1))
                        rh = stDs.tile([1, CW], f32, tag="hrow")
                        nc.scalar.activation(out=rh[:], in_=psh[:], func=SQRT,
                                             bias=eps_sb[0:1], scale=1.0 / D)
                        nc.vector.reciprocal(out=rh[:], in_=rh[:])
                        s2b = stDs.tile([1, CW], CDT, tag="s2b")
                        nc.vector.tensor_copy(out=s2b[:], in_=rh[:])
                        s2l = stDs.tile([P, CW], CDT, tag="s2l")
                        nc.gpsimd.partition_broadcast(s2l[:], s2b[:])
                        for kt in range(DT):
                            nc.vector.tensor_mul(hn_sb[:, kt, :],
                                                 hn_sb[:, kt, :], s2l[:])
                        g_sb = stDg.tile([P, FT, CW], CDT, tag="g")
                        for ft in range(FT):
                            w1t = stDw.tile([P, DT, P], CDT, tag="w1")
                            w3t = stDw.tile([P, DT, P], CDT, tag="w3")
                            nc.scalar.dma_start(out=w1t[:], in_=w_1[ft])
                            nc.sync.dma_start(out=w3t[:], in_=w_3[ft])
                            p1 = ps_f1.tile([P, CW], f32, tag="p1")
                            p3 = ps_f3.tile([P, CW], f32, tag="p3")
                            for kt in range(DT):
                                nc.tensor.matmul(p1[:], w1t[:, kt],
                                                 hn_sb[:, kt, :],
                                                 start=(kt == 0),
                                                 stop=(kt == DT - 1))
                            for kt in range(DT):
                                nc.tensor.matmul(p3[:], w3t[:, kt],
                                                 hn_sb[:, kt, :],
                                                 start=(kt == 0),
                                                 stop=(kt == DT - 1))
                            tsi = stDt.tile([P, CW], CDT, tag="tsi")
                            nc.scalar.activation(out=tsi[:], in_=p1[:],
                                                 func=SILU)
                            nc.vector.tensor_mul(g_sb[:, ft, :], tsi[:],
                                                 p3[:])
                        if c == 0:
                            h_block(2, hstD, xtpD)
                            h_block(3, hstD, xtpD)
                        else:
                            residual(c - 1)
                        for ot in range(32):
                            w2t = stDw2.tile([P, FT, P], CDT, tag="w2")
                            if ot % 2 == 0:
                                nc.scalar.dma_start(out=w2t[:], in_=w_2[ot])
                            else:
                                nc.sync.dma_start(out=w2t[:], in_=w_2[ot])
                            pt = ps_w2.tile([P, CW], f32, tag="pw2")
                            for ft in range(FT):
                                nc.tensor.matmul(pt[:], w2t[:, ft],
                                                 g_sb[:, ft, :],
                                                 start=(ft == 0),
                                                 stop=(ft == FT - 1))
                            og = stDt.tile([P, CW], CDT, tag="og")
                            if ot % 2 == 0:
                                nc.vector.tensor_copy(out=og[:], in_=pt[:])
                            else:
                                nc.scalar.copy(out=og[:], in_=pt[:])
                            nc.sync.dma_start(
                                out=foT_cc[c][P * ot:P * (ot + 1), :],
                                in_=og[:])
                        nc.gpsimd.collective_compute(
                            "ReduceScatter", ADD, ins=[foT_cc[c][:]],
                            outs=[fo_rs[c][:]], replica_groups=RG)
                        if c == NCH - 1:
                            residual(c)

    nc.compile()
    return nc


def _prep_inputs(x, freqs_cos, freqs_sin, mask, attn_norm_w, wq, wk, wv, wo,
                 ffn_norm_w, w1, w2, w3):
    """Host-side sharding + weight layout. Returns in_maps for 8 cores."""
    f32 = np.float32
    x2 = np.asarray(x, f32)[0]                     # [S, D]
    xT = np.ascontiguousarray(x2.T)                # [D, S]
    # SBUF-tile-ordered x: x_ch[c, p, kt, s] = xT[128*kt+p, 512*c+s]
    x_ch = np.ascontiguousarray(
        xT.astype(NP_CDT).reshape(DT, P, NCH, CW).transpose(2, 1, 0, 3))
    anw = np.asarray(attn_norm_w, f32)
    fnw = np.asarray(ffn_norm_w, f32)
    wq = np.asarray(wq, f32) * anw[None, :]
    wk = np.asarray(wk, f32) * anw[None, :]
    wv_e = np.asarray(wv, f32)
    wo = np.asarray(wo, f32)
    w1 = np.asarray(w1, f32) * fnw[None, :]
    w3 = np.asarray(w3, f32) * fnw[None, :]
    w2 = np.asarray(w2, f32)

    perm = np.concatenate([np.arange(0, HD, 2), np.arange(1, HD, 2)])

    cosT = np.ascontiguousarray(np.asarray(freqs_cos, f32).T)   # [64, S]
    sinT = np.ascontiguousarray(np.asarray(freqs_sin, f32).T)
    cos2 = np.concatenate([cosT, cosT], axis=0).astype(NP_CDT)  # [128, S]
    sinsg2 = np.concatenate([-sinT, sinT], axis=0).astype(NP_CDT)
    m = np.asarray(mask, f32)[0, 0]
    dmask = (np.ascontiguousarray(m[:P, :P].T) * f32(math.sqrt(HD))).astype(f32)

    def lhsT_tiles(wt, n_out_tiles, n_k_tiles):
        # wt: [K, Mout] -> [ot, p, kt, j] with [ot,p,kt,j] = wt[128*kt+p, 128*ot+j]
        a = wt.reshape(n_k_tiles, P, n_out_tiles, P)
        return np.ascontiguousarray(a.transpose(2, 1, 0, 3)).astype(NP_CDT)

    in_maps = []
    for r in range(CORES):
        ds = slice(DQ * r, DQ * (r + 1))
        wqT = wq[ds].T.copy()                      # [D, DQ]
        wkT = wk[ds].T.copy()
        for h in range(NHC):
            blk = slice(HD * h, HD * (h + 1))
            wqT[:, blk] = wqT[:, blk][:, perm]
            wkT[:, blk] = wkT[:, blk][:, perm]
        wqk = np.concatenate([lhsT_tiles(wqT, NHC, DT),
                              lhsT_tiles(wkT, NHC, DT)], axis=0)  # [8,P,DT,P]
        wvT = wv_e[ds].T.copy()                    # [D, DQ]
        # [P, DT, DQ]: 32KB contiguous per partition -> one efficient DMA
        w_v_l = np.ascontiguousarray(
            wvT.reshape(DT, P, DQ).transpose(1, 0, 2)).astype(NP_CDT)
        # wo ROW-sharded: contract own 512 attn dims, all 4096 out dims
        # w_o_l[p, ot, h, j] = wo[128*ot+j, 512*r + 128*h + p]
        a = np.ascontiguousarray(wo[:, ds].T)      # [512 d_own, 4096 o]
        w_o_l = np.ascontiguousarray(
            a.reshape(NHC, P, DT, P).transpose(1, 2, 0, 3)).astype(NP_CDT)
        fs = slice(FC * r, FC * (r + 1))
        w1s = np.zeros((FP, D), f32)
        w3s = np.zeros((FP, D), f32)
        w1s[:FC] = w1[fs]
        w3s[:FC] = w3[fs]
        w1_l = lhsT_tiles(np.ascontiguousarray(w1s.T), FT, DT)  # [FT, P, DT, P]
        w3_l = lhsT_tiles(np.ascontiguousarray(w3s.T), FT, DT)
        w2s = np.zeros((FP, D), f32)
        w2s[:FC] = w2[:, fs].T                     # [FP, D] (rows = f)
        w2_l = lhsT_tiles(w2s, 32, FT)             # [32, P, FT, P]

        in_maps.append({
            "xT_s": np.ascontiguousarray(xT[ds]),
            "x_ch": x_ch,
            "w_qk": wqk,
            "w_v": w_v_l,
            "w_o": w_o_l,
            "w_1": w1_l,
            "w_3": w3_l,
            "w_2": w2_l,
            "cos2": cos2,
            "sinsg2": sinsg2,
            "dmask": dmask,
        })
    return in_maps


def kernel(x, freqs_cos, freqs_sin, mask, attn_norm_w, wq, wk, wv, wo,
           ffn_norm_w, w1, w2, w3, _trace=False):
    global _COMPILED
    if _COMPILED is None:
        _COMPILED = _build()
    nc = _COMPILED
    in_maps = _prep_inputs(x, freqs_cos, freqs_sin, mask, attn_norm_w,
                           wq, wk, wv, wo, ffn_norm_w, w1, w2, w3)
    res = run_bass_kernel_spmd(nc, in_maps, list(range(CORES)), trace=_trace)
    kernel.last_result = res
    outT = np.concatenate([res.results[r]["outT_s"] for r in range(CORES)],
                          axis=0)                  # [D, S]
    return np.ascontiguousarray(outT.T)[None].astype(np.float32)
